# revision 34
# baseline (speedup 1.0000x reference)
"""Trainium2 Bass kernel for nn_CLEAR_46643344834801 (retrieval_knn SOM forward).

Computes, for x [131072, 64], prototypes/grid_pos [256, 64]:
    d_total = cdist(x, prototypes) + cdist(x, grid_pos)
    w = exp(-d_total/temp) -> row-normalize(+eps) -> *gate -> row-normalize(+eps)
    blended = w @ prototypes
Returns (blended [N,64] f32, w [N,256] f32).

Strategy: data-parallel over 8 NeuronCores (16384 points each), 128-point
tiles. The host ships x pre-transposed and augmented ([x | 1 | x2]^T, f32r)
so one K=66 matmul against [-2p | p2 | 1] yields the full squared distance
for both prototype sets straight from DMA (f32r streams at 1 cycle/row on
the PE). ACT does sqrt over two tiles per instruction and exp over four
(sqrt/exp live in different activation-table sets, so the kernel ping-pongs
between per-chunk phases with explicit same-engine ordering deps to bound
table reloads at 2 per chunk). GPSIMD adds the two distance halves. The exp
is shifted by a data-scaled bias into fp16 range so the e tensor, its PE
transposes, and the blend matmul all run at 2-byte cost; the shift cancels
exactly in the normalization. The blend matmul uses a ones-augmented
prototype matrix so its extra column yields the row sum S for free, and the
double eps-normalization collapses to w = e*gate/((gate+eps)*S + eps^2)
(uniform gate; the general-gate path keeps S and Seg separately). DMAs are
batched across 4-8 tiles, stores per exp-quad, with next-chunk x loads
prefetched ahead of this chunk's stores to avoid HWDGE head-of-line
blocking. Tail chunks taper (16/8/4 tiles) so the pipeline drains short.
"""

import os
import sys

import numpy as np

if "/opt/trn_rl_repo" not in sys.path:
    sys.path.insert(0, "/opt/trn_rl_repo")

N, D, C = 131072, 64, 256
NCORES = 8
EPS = 1e-8
TEMP_LO = 0.001
PTS = 128          # points per tile (SBUF partitions)
KAUG = D + 2       # augmented contraction dim: [x | 1 | x2]
DAUG = D + 1       # blend output: [blended | S]
F2 = 2 * C         # both distance halves side by side
XB = 8             # x-load / blended-store batch (tiles per DMA)
WB = 2             # transpose-copy pair
QB = 4             # exp batch / w-store batch (tiles)


def _chunk_plan(ntiles, chunk_tiles, tail):
    plan = []
    left = ntiles
    while left > 0:
        if left > chunk_tiles or not tail:
            take = min(chunk_tiles, left)
        else:
            take = None
            for t in tail:
                if left > t:
                    take = min(chunk_tiles, left - t)
                    break
            if take is None:
                take = left
        plan.append(take)
        left -= take
    return plan


def _build_program(npc, temp, gate0, eshift, chunk_tiles, uniform_gate,
                   tail=(16, 10, 4, 2)):
    import concourse.tile as tile
    from concourse import bacc, masks, mybir
    from concourse.bass import _add_dep_helper

    ntiles = npc // PTS
    assert chunk_tiles % QB == 0 and QB % WB == 0
    f32 = mybir.dt.float32
    f16 = mybir.dt.float16
    f32r = mybir.dt.float32r
    AF = mybir.ActivationFunctionType
    ALU = mybir.AluOpType

    nc = bacc.Bacc("TRN2", target_bir_lowering=False, debug=False,
                   num_devices=NCORES)

    xt_d = nc.dram_tensor("xaugt", [KAUG, npc], f32r,
                          kind="ExternalInput").ap()
    rhs_d = nc.dram_tensor("rhsaug", [KAUG, F2], f32r,
                           kind="ExternalInput").ap()
    pmat_d = nc.dram_tensor("pmataug", [C, DAUG], f16,
                            kind="ExternalInput").ap()
    gate_d = nc.dram_tensor("gaterep", [PTS, C], f32, kind="ExternalInput").ap()
    w_d = nc.dram_tensor("w_out", [npc, C], f32, kind="ExternalOutput").ap()
    bl_d = nc.dram_tensor("bl_out", [npc, D], f32, kind="ExternalOutput").ap()

    def dma_load(out, in_):
        return nc.sync.dma_start(out, in_)

    def dma_store(out, in_):
        return nc.sync.dma_start(out, in_)

    with tile.TileContext(nc) as tc:
        with (
            tc.tile_pool(name="const", bufs=1) as const_pool,
            tc.tile_pool(name="xt", bufs=6) as x_pool,
            tc.tile_pool(name="dsb", bufs=10) as d_pool,
            tc.tile_pool(name="ds", bufs=chunk_tiles // QB + 2) as dsum_pool,
            tc.tile_pool(name="ee", bufs=chunk_tiles // QB + 2) as e_pool,
            tc.tile_pool(name="ww", bufs=3) as w_pool,
            tc.tile_pool(name="wt", bufs=4) as wt_pool,
            tc.tile_pool(name="bl", bufs=3) as bl_pool,
            tc.tile_pool(name="sm", bufs=4) as sm_pool,
            tc.tile_pool(name="psD", bufs=2, space="PSUM") as psD_pool,
            tc.tile_pool(name="psW", bufs=2, space="PSUM") as psW_pool,
            tc.tile_pool(name="psB", bufs=2, space="PSUM") as psB_pool,
        ):
            # first x tiles before anything else so the HWDGE FIFO
            # serves them ahead of the constant loads
            xw_prefetched = {}
            xw0 = x_pool.tile([KAUG, XB * PTS], f32r, tag="xt")
            dma_load(xw0[:, 0:2 * PTS], xt_d[:, 0:2 * PTS])
            g0n = min(XB, ntiles)
            if g0n > 2:
                dma_load(xw0[:, 2 * PTS:g0n * PTS],
                         xt_d[:, 2 * PTS:g0n * PTS])
            xw_prefetched[(0, 0)] = xw0

            # ---- constants ----
            ident16 = const_pool.tile([PTS, PTS], f16)
            masks.make_identity(nc, ident16[:])

            eshift_sb = const_pool.tile([PTS, 1], f32)
            nc.vector.memset(eshift_sb[:], float(eshift / temp))

            rhs_sb = const_pool.tile([KAUG, F2], f32r)
            nc.sync.dma_start(rhs_sb[:], rhs_d[:])

            pmat_sb = []
            for h in range(2):
                t = const_pool.tile([PTS, DAUG], f16, tag=f"pmat{h}")
                nc.sync.dma_start(t[:], pmat_d[h * PTS:(h + 1) * PTS, :])
                pmat_sb.append(t)

            gate_sb = None
            if not uniform_gate:
                gate_sb = const_pool.tile([PTS, C], f32)
                nc.sync.dma_start(gate_sb[:], gate_d[:])

            plan = _chunk_plan(ntiles, chunk_tiles, tail)
            starts = [sum(plan[:i]) for i in range(len(plan))]
            prev_last_exp = None
            for ci in range(len(plan)):
                t0 = starts[ci]
                tcount = plan[ci]

                dtiles = []
                sqrt_insts = []
                # ---- phase A: distances + sqrt (sqrt table set) ----
                for jg in range(0, tcount, XB):
                    gb = min(XB, tcount - jg)
                    key = (ci, jg)
                    if key in xw_prefetched:
                        xw = xw_prefetched.pop(key)
                    else:
                        xw = x_pool.tile([KAUG, XB * PTS], f32r, tag="xt")
                        c0 = (t0 + jg) * PTS
                        if ci == 0 and jg == 0:
                            # split the very first load so the pipeline
                            # fills as soon as two tiles have landed
                            s0 = min(2, gb) * PTS
                            dma_load(xw[:, 0:s0], xt_d[:, c0:c0 + s0])
                            if gb * PTS > s0:
                                dma_load(xw[:, s0:gb * PTS],
                                         xt_d[:, c0 + s0:c0 + gb * PTS])
                        else:
                            dma_load(xw[:, 0:gb * PTS],
                                     xt_d[:, c0:c0 + gb * PTS])
                    for jp0 in range(jg, jg + gb, 2):
                        pn = min(2, jg + gb - jp0)
                        d2_ps = psD_pool.tile([PTS, 2 * F2], f32)
                        for p in range(pn):
                            lhs = xw[:, (jp0 - jg + p) * PTS:
                                     (jp0 - jg + p + 1) * PTS]
                            with tc.high_priority(offset=10000):
                                nc.tensor.matmul(
                                    d2_ps[:, p * F2:(p + 1) * F2], lhs,
                                    rhs_sb[:], start=True, stop=True)
                        dpair = d_pool.tile([PTS, 2 * F2], f32, tag="dtile")
                        sq = nc.scalar.activation(
                            dpair[:, 0:pn * F2], d2_ps[:, 0:pn * F2],
                            AF.Sqrt)
                        sqrt_insts.append(sq)
                        if prev_last_exp is not None:
                            _add_dep_helper(sq.ins, prev_last_exp.ins,
                                            sync=False,
                                            reason="act table phase order")
                        for p in range(pn):
                            dtiles.append(dpair[:, p * F2:(p + 1) * F2])

                # prefetch the next chunk's first x groups so their DMAs
                # are queued ahead of this chunk's stores
                nt0 = t0 + tcount
                if ci + 1 < len(plan):
                    ntc = plan[ci + 1]
                    for jg in range(0, min(ntc, 3 * XB), XB):
                        gb = min(XB, ntc - jg)
                        xw = x_pool.tile([KAUG, XB * PTS], f32r, tag="xt")
                        c0 = (nt0 + jg) * PTS
                        dma_load(xw[:, 0:gb * PTS], xt_d[:, c0:c0 + gb * PTS])
                        xw_prefetched[(ci + 1, jg)] = xw

                # ---- phase B: exp (quad-batched) + blend + normalize --
                denmat = sm_pool.tile([PTS, chunk_tiles], f32, tag="denmat")
                Smat = None
                if not uniform_gate:
                    Smat = sm_pool.tile([PTS, chunk_tiles], f32, tag="Smat")
                exp_insts = []
                blw = None
                bl_base = bl_n = 0
                for jq0 in range(0, tcount, QB):
                    qqn = min(QB, tcount - jq0)
                    dsum2 = dsum_pool.tile([PTS, QB * C], f32, tag="dsum")
                    for q in range(qqn):
                        dtile = dtiles[jq0 + q]
                        if q == QB - 1:
                            nc.vector.tensor_add(
                                dsum2[:, q * C:(q + 1) * C],
                                dtile[:, 0:C], dtile[:, C:F2])
                        else:
                            nc.gpsimd.tensor_tensor(
                                dsum2[:, q * C:(q + 1) * C],
                                dtile[:, 0:C], dtile[:, C:F2], ALU.add)
                    e2 = e_pool.tile([PTS, QB * C], f16, tag="e")
                    if uniform_gate:
                        ex = nc.scalar.activation(
                            e2[:, 0:qqn * C], dsum2[:, 0:qqn * C], AF.Exp,
                            scale=-1.0 / temp, bias=eshift_sb[:])
                        _add_dep_helper(ex.ins, sqrt_insts[-1].ins,
                                        sync=False,
                                        reason="act table phase order")
                        exp_insts.append(ex)
                    else:
                        for q in range(qqn):
                            j = jq0 + q
                            ex = nc.scalar.activation(
                                e2[:, q * C:(q + 1) * C],
                                dsum2[:, q * C:(q + 1) * C], AF.Exp,
                                scale=-1.0 / temp, bias=eshift_sb[:],
                                accum_out=Smat[:, j:j + 1])
                            _add_dep_helper(ex.ins, sqrt_insts[-1].ins,
                                            sync=False,
                                            reason="act table phase order")
                            exp_insts.append(ex)

                    wwide = w_pool.tile([PTS, QB * C], f32, tag="w")
                    for jq in range(jq0, jq0 + qqn, WB):
                        qn = min(WB, jq0 + qqn - jq)
                        es = []
                        bls = []
                        eT_ps = psW_pool.tile([PTS, 2 * WB * PTS], f16)
                        eT = wt_pool.tile([PTS, 2 * WB * PTS], f16, tag="wT")
                        for q in range(qn):
                            e = e2[:, (jq - jq0 + q) * C:
                                   (jq - jq0 + q + 1) * C]
                            if not uniform_gate:
                                eg = e_pool.tile([PTS, C], f16, tag="eg")
                                nc.vector.scalar_tensor_tensor(
                                    out=eg[:], in0=e, scalar=1.0,
                                    in1=gate_sb[:], op0=ALU.bypass,
                                    op1=ALU.mult)
                                e = eg[:]
                            es.append(e)
                            for h in range(2):
                                nc.tensor.transpose(
                                    eT_ps[:, (2 * q + h) * PTS:
                                          (2 * q + h + 1) * PTS],
                                    e[:, h * PTS:(h + 1) * PTS],
                                    ident16[:])
                        nc.vector.tensor_copy(
                            eT[:, 0:2 * qn * PTS], eT_ps[:, 0:2 * qn * PTS])
                        for q in range(qn):
                            j = jq + q
                            bl_ps = psB_pool.tile([PTS, DAUG], f32)
                            for h in range(2):
                                nc.tensor.matmul(
                                    bl_ps[:],
                                    eT[:, (2 * q + h) * PTS:
                                       (2 * q + h + 1) * PTS],
                                    pmat_sb[h][:], start=(h == 0),
                                    stop=(h == 1))
                            bls.append(bl_ps)
                            if uniform_gate:
                                # den = ((gate+eps)*S + eps^2) / gate
                                nc.vector.tensor_scalar(
                                    out=denmat[:, j:j + 1],
                                    in0=bl_ps[:, D:DAUG],
                                    scalar1=float((gate0 + EPS) / gate0),
                                    scalar2=float(
                                        EPS * EPS * np.exp(eshift / temp)
                                        / gate0),
                                    op0=ALU.mult, op1=ALU.add)
                            else:
                                # den = Seg + eps*S + eps^2 (Seg from the
                                # ones-column of the eg blend)
                                nc.vector.scalar_tensor_tensor(
                                    out=denmat[:, j:j + 1],
                                    in0=Smat[:, j:j + 1], scalar=EPS,
                                    in1=bl_ps[:, D:DAUG],
                                    op0=ALU.mult, op1=ALU.add)
                                nc.vector.tensor_scalar(
                                    out=denmat[:, j:j + 1],
                                    in0=denmat[:, j:j + 1],
                                    scalar1=float(
                                        EPS * EPS * np.exp(eshift / temp)),
                                    scalar2=None, op0=ALU.add)

                        inv = sm_pool.tile([PTS, WB], f32, tag="inv")
                        nc.vector.reciprocal(inv[:, 0:qn],
                                             denmat[:, jq:jq + qn])

                        for q in range(qn):
                            qq = jq - jq0 + q
                            nc.vector.tensor_scalar(
                                out=wwide[:, qq * C:(qq + 1) * C], in0=es[q],
                                scalar1=inv[:, q:q + 1], scalar2=None,
                                op0=ALU.mult)
                        if jq + qn == jq0 + qqn:
                            r0 = (t0 + jq0) * PTS
                            dma_store(
                                w_d[r0:r0 + qqn * PTS, :].rearrange(
                                    "(j p) c -> p j c", p=PTS),
                                wwide[:, 0:qqn * C].rearrange(
                                    "p (j c) -> p j c", j=qqn))
                        for q in range(qn):
                            j = jq + q
                            kb = j % XB
                            if kb == 0:
                                blw = bl_pool.tile([PTS, XB * D], f32,
                                                   tag="bl")
                                bl_base, bl_n = t0 + j, min(XB, tcount - j)
                            nc.vector.tensor_scalar(
                                out=blw[:, kb * D:(kb + 1) * D],
                                in0=bls[q][:, 0:D], scalar1=inv[:, q:q + 1],
                                scalar2=None, op0=ALU.mult)
                            if kb == bl_n - 1:
                                r0 = bl_base * PTS
                                dma_store(
                                    bl_d[r0:r0 + bl_n * PTS, :].rearrange(
                                        "(j p) d -> p j d", p=PTS),
                                    blw[:, 0:bl_n * D].rearrange(
                                        "p (j d) -> p j d", j=bl_n))
                prev_last_exp = exp_insts[-1]

    nc.compile()
    return nc


def _host_prep(prototypes, grid_pos, gate_logits, temp_raw):
    p64 = prototypes.astype(np.float64)
    g64 = grid_pos.astype(np.float64)
    temp = float(1.0 / (1.0 + np.exp(-float(temp_raw[0]))) * (1.0 - TEMP_LO)
                 + TEMP_LO)
    gate = (1.0 / (1.0 + np.exp(-gate_logits.astype(np.float64)))).astype(
        np.float32)
    rhs_aug = np.zeros((KAUG, F2), np.float32)
    rhs_aug[0:D, 0:C] = (-2.0 * p64.T).astype(np.float32)
    rhs_aug[0:D, C:F2] = (-2.0 * g64.T).astype(np.float32)
    rhs_aug[D, 0:C] = np.sum(
        prototypes.astype(np.float32) ** 2, axis=1, dtype=np.float32)
    rhs_aug[D, C:F2] = np.sum(
        grid_pos.astype(np.float32) ** 2, axis=1, dtype=np.float32)
    rhs_aug[D + 1, :] = 1.0
    pmat_aug = np.ones((C, DAUG), np.float16)
    pmat_aug[:, 0:D] = prototypes.astype(np.float16)
    gate_rep = np.broadcast_to(gate[None, :], (PTS, C)).copy()
    uniform = bool(np.all(gate == gate[0]))
    return temp, float(gate[0]), rhs_aug, pmat_aug, gate_rep, uniform


def kernel(x, prototypes, grid_pos, gate_logits, temp_raw):
    from concourse.bass_utils import run_bass_kernel_spmd

    x = np.ascontiguousarray(x, np.float32)
    npc = x.shape[0] // NCORES
    ntiles = npc // PTS
    temp, gate0, rhs_aug, pmat_aug, gate_rep, uniform = _host_prep(
        prototypes, grid_pos, gate_logits, temp_raw)

    x2 = np.sum(x * x, axis=1, dtype=np.float32)
    # Shift exp into fp16 range: m ~ typical total distance, minus margin.
    p2m = float(np.median(rhs_aug[D, 0:C]))
    g2m = float(np.median(rhs_aug[D, C:F2]))
    x2m = float(np.median(x2))
    eshift = float(np.sqrt(max(x2m + p2m, 0.0))
                   + np.sqrt(max(x2m + g2m, 0.0)) - 2.0)
    nc = _build_program(npc, temp, gate0, eshift, chunk_tiles=32,
                        uniform_gate=uniform)

    in_maps = []
    for i in range(NCORES):
        xs = x[i * npc:(i + 1) * npc]
        xaugt = np.ones((KAUG, npc), np.float32)
        xaugt[0:D, :] = xs.T
        xaugt[D + 1, :] = x2[i * npc:(i + 1) * npc]
        in_maps.append({
            "xaugt": xaugt,
            "rhsaug": rhs_aug,
            "pmataug": pmat_aug,
            "gaterep": gate_rep,
        })
    res = run_bass_kernel_spmd(nc, in_maps, list(range(NCORES)))
    w = np.concatenate([res.results[i]["w_out"] for i in range(NCORES)], axis=0)
    blended = np.concatenate(
        [res.results[i]["bl_out"] for i in range(NCORES)], axis=0)
    return blended.astype(np.float32), w.astype(np.float32)
